# revision 8
# baseline (speedup 1.0000x reference)
"""FlowNetC correlation (max_displacement=20, stride2=2) on 8 trn2 NeuronCores.

Strategy: data-parallel over batch (B=8 -> 1 batch/core). Per core, the
cost volume out[d,y,x] = (1/C) sum_c in1[c,y,x]*in2p[c,y+oy,x+ox] is
computed as a banded Gram matrix on the tensor engine:

  - stationary (lhsT): 128 in1 feature vectors at an 8x16 grid of
    positions (y0+2i, x0+2j)  [one (y,x)-parity class, spacing 2 to
    match the displacement stride]
  - moving (rhs): in2p feature vectors over the 28x36 window
    (y0+2kr, x0+2ks), kr<28, ks<36
  - psum[m=(i,j), n=(kr,ks)] = dot(in1_m, in2p_n); the 441 useful
    displacements for position m sit at kr in [i,i+20], ks in [j,j+20].

The slightly-overcomplete [y,x,28*36] tensor is written to DRAM with
fully-regular access patterns (the band extraction is an inherently
per-partition shear no engine/DMA AP can express); the final 21x21
window slice per position happens on host in numpy. All device I/O is
bf16 (inputs pre-scaled by 1/C=2^-8 exactly on host; psum accumulates
in fp32).
"""

import numpy as np
import ml_dtypes

B, C, H, W = 8, 256, 96, 128
MAXD = 20  # pad size
PH, PW = H + 2 * MAXD, W + 2 * MAXD  # 136, 168
A_, B_ = 8, 16  # stationary grid (rows i, cols j)
KR, KS = A_ + MAXD, B_ + MAXD  # 28, 36 moving window
NF = KR * KS  # 1008 psum free size
N_CORES = 8

_cached = None


def _split_multiwait(nc):
    """This walrus build accepts at most one sem-wait per instruction.
    Move extra waits onto standalone EventSemaphore carriers inserted
    just before the instruction (same engine => program order holds)."""
    import concourse.mybir as mybir

    n = 0
    for f in nc.m.functions:
        for bb in f.blocks:
            insts = bb.instructions
            i = 0
            while i < len(insts):
                inst = insts[i]
                si = inst.sync_info
                if si is not None and si.on_wait and len(si.on_wait) > 1:
                    waits = list(si.on_wait)
                    si.on_wait = waits[-1:]
                    for w in waits[:-1]:
                        car = mybir.InstEventSemaphore(
                            name=f"WSPLIT-{n}", ins=[], outs=[]
                        )
                        n += 1
                        car.engine = inst.engine
                        car.sync_info = type(si)(on_wait=[w], on_update=[])
                        insts.insert(i, car)
                        i += 1
                i += 1
    return n


def _build():
    import concourse.bass as bass
    import concourse.mybir as mybir
    import concourse.tile as tile

    bf16 = mybir.dt.bfloat16
    f32 = mybir.dt.float32

    nc = bass.Bass("TRN2", target_bir_lowering=False, debug=False)
    # x1 arrives host-packed: [c, tile, m] with tile=(ty,tx,py,px), m=(i,j)
    x1 = nc.dram_tensor("x1", [C, H, W], bf16, kind="ExternalInput").ap()
    x2 = nc.dram_tensor("x2", [C, H, W], bf16, kind="ExternalInput").ap()
    z = nc.dram_tensor("z", [H, W, NF], bf16, kind="ExternalOutput").ap()

    with tile.TileContext(nc) as tc:
        with (
            tc.tile_pool(name="resident", bufs=1) as rpool,
            tc.tile_pool(name="psum", bufs=4, space="PSUM") as ppool,
            tc.tile_pool(name="s2", bufs=6) as spool,
        ):
            a_sb = []  # in1 chunks [128, H*W]
            p_sb = []  # padded in2 chunks [128, PH*PW]
            for k in range(2):
                a = rpool.tile([128, H * W], bf16, tag=f"a{k}")
                a_sb.append(a)
                p = rpool.tile([128, PH * PW], bf16, tag=f"p{k}")
                p_sb.append(p)

            for k in range(2):
                p3 = p_sb[k][:].rearrange("p (r s) -> p r s", r=PH, s=PW)
                # zero borders: top rows, bottom rows, left/right cols
                nc.vector.memset(p3[:, 0:MAXD, :], 0.0)
                nc.vector.memset(p3[:, PH - MAXD : PH, :], 0.0)
                nc.vector.memset(p3[:, MAXD : PH - MAXD, 0:MAXD], 0.0)
                nc.vector.memset(p3[:, MAXD : PH - MAXD, PW - MAXD : PW], 0.0)
                # interior load
                nc.sync.dma_start(
                    p3[:, MAXD : PH - MAXD, MAXD : MAXD + W],
                    x2[k * 128 : (k + 1) * 128, :, :],
                )
                nc.sync.dma_start(
                    a_sb[k][:],
                    x1[k * 128 : (k + 1) * 128, :, :].rearrange(
                        "c h w -> c (h w)"
                    ),
                )

            # views for parity-strided slicing
            p_v = [
                p_sb[k][:].rearrange(
                    "p (rq rp sq sp) -> p rq rp sq sp", rp=2, sp=2, sq=PW // 2
                )
                for k in range(2)
            ]
            z_v = z.rearrange(
                "(yq yp) (xq xp) n -> yq yp xq xp n", yp=2, xp=2
            )

            t_idx = 0
            for ty in range(H // 16):
                for tx in range(W // 32):
                    for py in range(2):
                        for px in range(2):
                            # stationary grid rows y=16ty+py+2i, cols x=32tx+px+2j
                            lhs = [
                                a_sb[k][:, t_idx * 128 : (t_idx + 1) * 128]
                                for k in range(2)
                            ]
                            t_idx += 1
                            ps = [
                                ppool.tile([128, NF // 2], f32, name=f"ps{h}", tag=f"ps{h}")
                                for h in range(2)
                            ]
                            for k in range(2):
                                for h in range(2):
                                    rhs = p_v[k][
                                        :,
                                        8 * ty + 14 * h : 8 * ty + 14 * (h + 1),
                                        py,
                                        16 * tx : 16 * tx + KS,
                                        px,
                                    ]
                                    nc.tensor.matmul(
                                        ps[h][:],
                                        lhs[k],
                                        rhs,
                                        start=(k == 0),
                                        stop=(k == 1),
                                    )
                            s2 = spool.tile([128, NF], bf16)
                            nc.vector.tensor_copy(s2[:, 0 : NF // 2], ps[0][:])
                            nc.scalar.copy(s2[:, NF // 2 : NF], ps[1][:])
                            nc.sync.dma_start(
                                z_v[
                                    8 * ty : 8 * ty + A_,
                                    py,
                                    16 * tx : 16 * tx + B_,
                                    px,
                                    :,
                                ],
                                s2[:],
                            )

    _split_multiwait(nc)
    return nc


def kernel(input1, input2):
    global _cached
    from concourse import bass_utils

    if _cached is None:
        _cached = _build()
    nc = _cached

    # exact 1/C scale (2^-8) folded into in1 before the bf16 rounding
    x1 = (input1 * np.float32(1.0 / C)).astype(ml_dtypes.bfloat16)
    # pack stationary tiles contiguously: [c, (ty,tx,py,px), (i,j)]
    x1 = np.ascontiguousarray(
        x1.reshape(B, C, 6, A_, 2, 4, B_, 2).transpose(0, 1, 2, 5, 4, 7, 3, 6)
    ).reshape(B, C, H, W)
    x2 = input2.astype(ml_dtypes.bfloat16)
    in_maps = [{"x1": x1[b], "x2": x2[b]} for b in range(N_CORES)]
    res = bass_utils.run_bass_kernel_spmd(
        nc, in_maps, core_ids=list(range(N_CORES))
    )
    Z = np.stack([res.results[b]["z"] for b in range(N_CORES)])
    Zf = Z.astype(np.float32).reshape(B, H, W, KR, KS)

    D = 21
    out = np.empty((B, D * D, H, W), np.float32)
    for yy in range(16):
        i = yy // 2
        for xx in range(32):
            j = xx // 2
            blk = Zf[:, yy::16, xx::32, i : i + D, j : j + D]
            out[:, :, yy::16, xx::32] = blk.reshape(
                B, H // 16, W // 32, D * D
            ).transpose(0, 3, 1, 2)
    return out


# revision 10
# speedup vs baseline: 1.0133x; 1.0133x over previous
"""FlowNetC correlation (max_displacement=20, stride2=2) on 8 trn2 NeuronCores.

Strategy: data-parallel over batch (B=8 -> 1 batch/core). Per core, the
cost volume out[d,y,x] = (1/C) sum_c in1[c,y,x]*in2p[c,y+oy,x+ox] is
computed as a banded Gram matrix on the tensor engine:

  - stationary (lhsT): 128 in1 feature vectors at an 8x16 grid of
    positions (y0+2i, x0+2j)  [one (y,x)-parity class, spacing 2 to
    match the displacement stride]
  - moving (rhs): in2p feature vectors over the 28x36 window
    (y0+2kr, x0+2ks), kr<28, ks<36
  - psum[m=(i,j), n=(kr,ks)] = dot(in1_m, in2p_n); the 441 useful
    displacements for position m sit at kr in [i,i+20], ks in [j,j+20].

The slightly-overcomplete [y,x,28*36] tensor is written to DRAM with
fully-regular access patterns (the band extraction is an inherently
per-partition shear no engine/DMA AP can express); the final 21x21
window slice per position happens on host in numpy. All device I/O is
bf16 (inputs pre-scaled by 1/C=2^-8 exactly on host; psum accumulates
in fp32).
"""

import numpy as np
import ml_dtypes

B, C, H, W = 8, 256, 96, 128
MAXD = 20  # pad size
PH, PW = H + 2 * MAXD, W + 2 * MAXD  # 136, 168
A_, B_ = 8, 16  # stationary grid (rows i, cols j)
KR, KS = A_ + MAXD, B_ + MAXD  # 28, 36 moving window
NF = KR * KS  # 1008 psum free size
N_CORES = 8

_cached = None


def _split_multiwait(nc):
    """This walrus build accepts at most one sem-wait per instruction.
    Move extra waits onto standalone EventSemaphore carriers inserted
    just before the instruction (same engine => program order holds)."""
    import concourse.mybir as mybir

    n = 0
    for f in nc.m.functions:
        for bb in f.blocks:
            insts = bb.instructions
            i = 0
            while i < len(insts):
                inst = insts[i]
                si = inst.sync_info
                if si is not None and si.on_wait and len(si.on_wait) > 1:
                    waits = list(si.on_wait)
                    si.on_wait = waits[-1:]
                    for w in waits[:-1]:
                        car = mybir.InstEventSemaphore(
                            name=f"WSPLIT-{n}", ins=[], outs=[]
                        )
                        n += 1
                        car.engine = inst.engine
                        car.sync_info = type(si)(on_wait=[w], on_update=[])
                        insts.insert(i, car)
                        i += 1
                i += 1
    return n


def _build():
    import concourse.bass as bass
    import concourse.mybir as mybir
    import concourse.tile as tile

    bf16 = mybir.dt.bfloat16
    f32 = mybir.dt.float32

    nc = bass.Bass("TRN2", target_bir_lowering=False, debug=False)
    # x1 arrives host-packed: [c, tile, m] with tile=(ty,tx,py,px), m=(i,j)
    x1 = nc.dram_tensor("x1", [C, H, W], bf16, kind="ExternalInput").ap()
    x2 = nc.dram_tensor("x2", [C, H, W], bf16, kind="ExternalInput").ap()
    z = nc.dram_tensor("z", [H, W, NF], bf16, kind="ExternalOutput").ap()

    with tile.TileContext(nc) as tc:
        with (
            tc.tile_pool(name="resident", bufs=1) as rpool,
            tc.tile_pool(name="psum", bufs=4, space="PSUM") as ppool,
            tc.tile_pool(name="s2", bufs=6) as spool,
        ):
            a_sb = []  # in1 chunks [128, H*W]
            p_sb = []  # padded in2 chunks [128, PH*PW]
            for k in range(2):
                a = rpool.tile([128, H * W], bf16, tag=f"a{k}")
                a_sb.append(a)
                p = rpool.tile([128, PH * PW], bf16, tag=f"p{k}")
                p_sb.append(p)

            for k in range(2):
                p3 = p_sb[k][:].rearrange("p (r s) -> p r s", r=PH, s=PW)
                # zero borders: top rows, bottom rows, left/right cols
                nc.vector.memset(p3[:, 0:MAXD, :], 0.0)
                nc.vector.memset(p3[:, PH - MAXD : PH, :], 0.0)
                nc.vector.memset(p3[:, MAXD : PH - MAXD, 0:MAXD], 0.0)
                nc.vector.memset(p3[:, MAXD : PH - MAXD, PW - MAXD : PW], 0.0)
            # band-split interior loads so early tiles' matmuls only wait
            # on the first bands; alternate the two HWDGE rings
            NB = 6
            hb = H // NB
            for b in range(NB):
                for k in range(2):
                    p3 = p_sb[k][:].rearrange("p (r s) -> p r s", r=PH, s=PW)
                    eng = nc.sync if (b + k) % 2 == 0 else nc.scalar
                    eng.dma_start(
                        p3[:, MAXD + b * hb : MAXD + (b + 1) * hb, MAXD : MAXD + W],
                        x2[k * 128 : (k + 1) * 128, b * hb : (b + 1) * hb, :],
                    )
                    eng2 = nc.scalar if (b + k) % 2 == 0 else nc.sync
                    eng2.dma_start(
                        a_sb[k][:, b * (hb * W) : (b + 1) * (hb * W)],
                        x1[k * 128 : (k + 1) * 128, b * hb : (b + 1) * hb, :].rearrange(
                            "c h w -> c (h w)"
                        ),
                    )

            # views for parity-strided slicing
            p_v = [
                p_sb[k][:].rearrange(
                    "p (rq rp sq sp) -> p rq rp sq sp", rp=2, sp=2, sq=PW // 2
                )
                for k in range(2)
            ]
            z_v = z.rearrange(
                "(yq yp) (xq xp) n -> yq yp xq xp n", yp=2, xp=2
            )

            t_idx = 0
            for ty in range(H // 16):
                for tx in range(W // 32):
                    for py in range(2):
                        for px in range(2):
                            # stationary grid rows y=16ty+py+2i, cols x=32tx+px+2j
                            lhs = [
                                a_sb[k][:, t_idx * 128 : (t_idx + 1) * 128]
                                for k in range(2)
                            ]
                            t_idx += 1
                            ps = [
                                ppool.tile([128, NF // 2], f32, name=f"ps{h}", tag=f"ps{h}")
                                for h in range(2)
                            ]
                            for k in range(2):
                                for h in range(2):
                                    rhs = p_v[k][
                                        :,
                                        8 * ty + 14 * h : 8 * ty + 14 * (h + 1),
                                        py,
                                        16 * tx : 16 * tx + KS,
                                        px,
                                    ]
                                    nc.tensor.matmul(
                                        ps[h][:],
                                        lhs[k],
                                        rhs,
                                        start=(k == 0),
                                        stop=(k == 1),
                                    )
                            s2 = spool.tile([128, NF], bf16)
                            nc.vector.tensor_copy(s2[:, 0 : NF // 2], ps[0][:])
                            nc.scalar.copy(s2[:, NF // 2 : NF], ps[1][:])
                            out_eng = nc.sync if t_idx % 2 == 0 else nc.scalar
                            out_eng.dma_start(
                                z_v[
                                    8 * ty : 8 * ty + A_,
                                    py,
                                    16 * tx : 16 * tx + B_,
                                    px,
                                    :,
                                ],
                                s2[:],
                            )

    _split_multiwait(nc)
    return nc


def kernel(input1, input2):
    global _cached
    from concourse import bass_utils

    if _cached is None:
        _cached = _build()
    nc = _cached

    # exact 1/C scale (2^-8) folded into in1 before the bf16 rounding
    x1 = (input1 * np.float32(1.0 / C)).astype(ml_dtypes.bfloat16)
    # pack stationary tiles contiguously: [c, (ty,tx,py,px), (i,j)]
    x1 = np.ascontiguousarray(
        x1.reshape(B, C, 6, A_, 2, 4, B_, 2).transpose(0, 1, 2, 5, 4, 7, 3, 6)
    ).reshape(B, C, H, W)
    x2 = input2.astype(ml_dtypes.bfloat16)
    in_maps = [{"x1": x1[b], "x2": x2[b]} for b in range(N_CORES)]
    res = bass_utils.run_bass_kernel_spmd(
        nc, in_maps, core_ids=list(range(N_CORES))
    )
    Z = np.stack([res.results[b]["z"] for b in range(N_CORES)])
    Zf = Z.astype(np.float32).reshape(B, H, W, KR, KS)

    D = 21
    out = np.empty((B, D * D, H, W), np.float32)
    for yy in range(16):
        i = yy // 2
        for xx in range(32):
            j = xx // 2
            blk = Zf[:, yy::16, xx::32, i : i + D, j : j + D]
            out[:, :, yy::16, xx::32] = blk.reshape(
                B, H // 16, W // 32, D * D
            ).transpose(0, 3, 1, 2)
    return out


# revision 11
# speedup vs baseline: 1.3859x; 1.3677x over previous
"""FlowNetC correlation (max_displacement=20, stride2=2) on 8 trn2 NeuronCores.

Strategy: data-parallel over batch (B=8 -> 1 batch/core). Per core, the
cost volume out[d,y,x] = (1/C) sum_c in1[c,y,x]*in2p[c,y+oy,x+ox] is
computed as a banded Gram matrix on the tensor engine:

  - stationary (lhsT): 128 in1 feature vectors at an 8x16 grid of
    positions (y0+2i, x0+2j)  [one (y,x)-parity class, spacing 2 to
    match the displacement stride]
  - moving (rhs): in2p feature vectors over the 28x36 window
    (y0+2kr, x0+2ks), kr<28, ks<36
  - psum[m=(i,j), n=(kr,ks)] = dot(in1_m, in2p_n); the 441 useful
    displacements for position m sit at kr in [i,i+20], ks in [j,j+20].

The slightly-overcomplete [y,x,28*36] tensor is written to DRAM with
fully-regular access patterns (the band extraction is an inherently
per-partition shear no engine/DMA AP can express); the final 21x21
window slice per position happens on host in numpy. All device I/O is
bf16 (inputs pre-scaled by 1/C=2^-8 exactly on host; psum accumulates
in fp32).
"""

import numpy as np
import ml_dtypes

B, C, H, W = 8, 256, 96, 128
MAXD = 20  # pad size
PH, PW = H + 2 * MAXD, W + 2 * MAXD  # 136, 168
A_, B_ = 16, 8  # stationary grid (rows i, cols j)
KR, KS = A_ + MAXD, B_ + MAXD  # 28, 36 moving window
NF = KR * KS  # 1008 psum free size
N_CORES = 8

_cached = None


def _split_multiwait(nc):
    """This walrus build accepts at most one sem-wait per instruction.
    Move extra waits onto standalone EventSemaphore carriers inserted
    just before the instruction (same engine => program order holds)."""
    import concourse.mybir as mybir

    n = 0
    for f in nc.m.functions:
        for bb in f.blocks:
            insts = bb.instructions
            i = 0
            while i < len(insts):
                inst = insts[i]
                si = inst.sync_info
                if si is not None and si.on_wait and len(si.on_wait) > 1:
                    waits = list(si.on_wait)
                    si.on_wait = waits[-1:]
                    for w in waits[:-1]:
                        car = mybir.InstEventSemaphore(
                            name=f"WSPLIT-{n}", ins=[], outs=[]
                        )
                        n += 1
                        car.engine = inst.engine
                        car.sync_info = type(si)(on_wait=[w], on_update=[])
                        insts.insert(i, car)
                        i += 1
                i += 1
    return n


def _build():
    import concourse.bass as bass
    import concourse.mybir as mybir
    import concourse.tile as tile

    bf16 = mybir.dt.bfloat16
    f32 = mybir.dt.float32

    nc = bass.Bass("TRN2", target_bir_lowering=False, debug=False)
    # x1 arrives host-packed: [c, tile, m] with tile=(ty,tx,py,px), m=(i,j)
    x1 = nc.dram_tensor("x1", [C, H, W], bf16, kind="ExternalInput").ap()
    x2 = nc.dram_tensor("x2", [C, H, W], bf16, kind="ExternalInput").ap()
    z = nc.dram_tensor("z", [H, W, NF], bf16, kind="ExternalOutput").ap()

    with tile.TileContext(nc) as tc:
        with (
            tc.tile_pool(name="resident", bufs=1) as rpool,
            tc.tile_pool(name="psum", bufs=4, space="PSUM") as ppool,
            tc.tile_pool(name="s2", bufs=6) as spool,
        ):
            a_sb = []  # in1 chunks [128, H*W]
            p_sb = []  # padded in2 chunks [128, PH*PW]
            for k in range(2):
                a = rpool.tile([128, H * W], bf16, tag=f"a{k}")
                a_sb.append(a)
                p = rpool.tile([128, PH * PW], bf16, tag=f"p{k}")
                p_sb.append(p)

            for k in range(2):
                p3 = p_sb[k][:].rearrange("p (r s) -> p r s", r=PH, s=PW)
                # zero borders: top rows, bottom rows, left/right cols
                nc.vector.memset(p3[:, 0:MAXD, :], 0.0)
                nc.vector.memset(p3[:, PH - MAXD : PH, :], 0.0)
                nc.vector.memset(p3[:, MAXD : PH - MAXD, 0:MAXD], 0.0)
                nc.vector.memset(p3[:, MAXD : PH - MAXD, PW - MAXD : PW], 0.0)
            # band-split interior loads so early tiles' matmuls only wait
            # on the first bands; alternate the two HWDGE rings
            NB = 6
            hb = H // NB
            for b in range(NB):
                for k in range(2):
                    p3 = p_sb[k][:].rearrange("p (r s) -> p r s", r=PH, s=PW)
                    eng = nc.sync if (b + k) % 2 == 0 else nc.scalar
                    eng.dma_start(
                        p3[:, MAXD + b * hb : MAXD + (b + 1) * hb, MAXD : MAXD + W],
                        x2[k * 128 : (k + 1) * 128, b * hb : (b + 1) * hb, :],
                    )
                    eng2 = nc.scalar if (b + k) % 2 == 0 else nc.sync
                    eng2.dma_start(
                        a_sb[k][:, b * (hb * W) : (b + 1) * (hb * W)],
                        x1[k * 128 : (k + 1) * 128, b * hb : (b + 1) * hb, :].rearrange(
                            "c h w -> c (h w)"
                        ),
                    )

            # views for parity-strided slicing
            p_v = [
                p_sb[k][:].rearrange(
                    "p (rq rp sq sp) -> p rq rp sq sp", rp=2, sp=2, sq=PW // 2
                )
                for k in range(2)
            ]
            z_v = z.rearrange(
                "(yq yp) (xq xp) n -> yq yp xq xp n", yp=2, xp=2
            )

            t_idx = 0
            for ty in range(H // (2 * A_)):
                for tx in range(W // (2 * B_)):
                    for py in range(2):
                        for px in range(2):
                            # stationary grid rows y=16ty+py+2i, cols x=32tx+px+2j
                            lhs = [
                                a_sb[k][:, t_idx * 128 : (t_idx + 1) * 128]
                                for k in range(2)
                            ]
                            t_idx += 1
                            ps = [
                                ppool.tile([128, NF // 2], f32, name=f"ps{h}", tag=f"ps{h}")
                                for h in range(2)
                            ]
                            for k in range(2):
                                for h in range(2):
                                    rhs = p_v[k][
                                        :,
                                        A_ * ty + (KR // 2) * h : A_ * ty + (KR // 2) * (h + 1),
                                        py,
                                        B_ * tx : B_ * tx + KS,
                                        px,
                                    ]
                                    nc.tensor.matmul(
                                        ps[h][:],
                                        lhs[k],
                                        rhs,
                                        start=(k == 0),
                                        stop=(k == 1),
                                    )
                            s2 = spool.tile([128, NF], bf16)
                            nc.vector.tensor_copy(s2[:, 0 : NF // 2], ps[0][:])
                            nc.scalar.copy(s2[:, NF // 2 : NF], ps[1][:])
                            out_eng = nc.sync if t_idx % 2 == 0 else nc.scalar
                            out_eng.dma_start(
                                z_v[
                                    A_ * ty : A_ * ty + A_,
                                    py,
                                    B_ * tx : B_ * tx + B_,
                                    px,
                                    :,
                                ],
                                s2[:],
                            )

    _split_multiwait(nc)
    return nc


def kernel(input1, input2):
    global _cached
    from concourse import bass_utils

    if _cached is None:
        _cached = _build()
    nc = _cached

    # exact 1/C scale (2^-8) folded into in1 before the bf16 rounding
    x1 = (input1 * np.float32(1.0 / C)).astype(ml_dtypes.bfloat16)
    # pack stationary tiles contiguously: [c, (ty,tx,py,px), (i,j)]
    x1 = np.ascontiguousarray(
        x1.reshape(B, C, H // (2 * A_), A_, 2, W // (2 * B_), B_, 2).transpose(0, 1, 2, 5, 4, 7, 3, 6)
    ).reshape(B, C, H, W)
    x2 = input2.astype(ml_dtypes.bfloat16)
    in_maps = [{"x1": x1[b], "x2": x2[b]} for b in range(N_CORES)]
    res = bass_utils.run_bass_kernel_spmd(
        nc, in_maps, core_ids=list(range(N_CORES))
    )
    Z = np.stack([res.results[b]["z"] for b in range(N_CORES)])
    Zf = Z.astype(np.float32).reshape(B, H, W, KR, KS)

    D = 21
    out = np.empty((B, D * D, H, W), np.float32)
    ystep, xstep = 2 * A_, 2 * B_
    for yy in range(ystep):
        i = yy // 2
        for xx in range(xstep):
            j = xx // 2
            blk = Zf[:, yy::ystep, xx::xstep, i : i + D, j : j + D]
            out[:, :, yy::ystep, xx::xstep] = blk.reshape(
                B, H // ystep, W // xstep, D * D
            ).transpose(0, 3, 1, 2)
    return out
